# revision 23
# baseline (speedup 1.0000x reference)
"""AttentionMV pooling kernel for Trainium2 (Bass/Tile), 8-core hybrid-sharded.

Computes, for full inputs x:(64,2048,1024) c:(64,1024) W:(1024,1) b:(2048,1)
U:(1024,2048):
    et = c @ U + (x @ W)[..., 0] + b[:, 0]        # (B, T)
    at = softmax(et, axis=-1)
    out = einsum('bt,bte->be', at, x)             # (B, E)

Sharding: 4-way over T x 2-way over B. Core k = (ts, bs) handles t-slice
ts (512 timesteps) for 32 batches, returning partial weighted sums and
partial softmax denominators (exp uses a fixed shift, so partials combine
exactly on the host; no collectives). T-sharding shrinks the replicated-U
read to 2 MiB/core; x is read exactly once. TL=512 makes per-partition
HBM reads 4 contiguous rows = 16 KiB, the fattest descriptors this layout
admits.

Per-core dataflow:
  1. x batches alternate between f32 loads (HWDGE on the sync queue; typed
     f32r for the PE) and bf16 cast-loads (SWDGE); splitting the stream
     between the two DGE paths keeps the SWDGE descriptor-ring SBUF traffic
     away from half the stream and issues from two queues in parallel.
  2. ct[t, b] = sum_e U[e,t] c[b,e] + bias[t] - SHIFT on PE (c transposed
     on-chip via identity matmuls), e on partitions via the natural U layout.
  3. et chunks (x . W reduced over e) on DVE scalar_tensor_tensor with
     accum_out (~1.2 us per 128x1024 chunk; GpSimd is avoided - its SBUF
     port is shared with the DVE and elementwise work there throttles both).
  4. exp(et + ct) on ACT into persistent ev tiles (f32r/bf16 by parity to
     match the x dtype - the PE rejects mixed 32/16-bit operands); weighted
     sums via accumulating PE matmuls (ev stationary, x streaming).
  5. Denominator: two final matmuls ones^T @ ev over all batches; host
     combines. Output copies lag one batch and stores two so the in-order
     ACT/Sync queues never wait on the current batch's work.
"""

import os

import numpy as np

import concourse.bass as bass
import concourse.mybir as mybir
import concourse.tile as tile
from concourse import bacc

B, T, E = 64, 2048, 1024
NCORES = 8
T_SHARD = 4
B_SHARD = 2
TL = T // T_SHARD  # local timesteps per core (512)
BL = B // B_SHARD  # local batches per core (32)
P = 128
NCH = TL // P  # 4 t-chunks of 128 per batch
NE = E // P  # 8 e-chunks
XBUFS = 6  # x tiles in flight per dtype lane (1 batch per tile)
F32 = mybir.dt.float32
F32R = mybir.dt.float32r
BF16 = mybir.dt.bfloat16
SHIFT = 10.0  # softmax exp shift; cancels exactly in the normalization

_CACHE = {}


def build_bass():
    nc = bacc.Bacc(None, target_bir_lowering=False)

    # host-derived inputs: W pre-broadcast (both dtypes), bias pre-shifted
    # and pre-arranged, c pre-transposed, U pre-arranged+cast — all plain
    # HWDGE loads, so no on-chip setup work gates the pipeline start
    x = nc.dram_tensor("x", [BL, TL, E], F32, kind="ExternalInput")
    wbf = nc.dram_tensor("wbf", [P, E], F32, kind="ExternalInput")
    wbb = nc.dram_tensor("wbb", [P, E], BF16, kind="ExternalInput")
    bias = nc.dram_tensor("b", [P, NCH], F32, kind="ExternalInput")
    cT_in = nc.dram_tensor("ct", [P, NE, BL], BF16, kind="ExternalInput")
    U_in = nc.dram_tensor("u", [P, NE, TL], BF16, kind="ExternalInput")
    out = nc.dram_tensor("out", [BL, E], F32, kind="ExternalOutput")
    den_out = nc.dram_tensor("den", [1, BL * NCH], F32, kind="ExternalOutput")

    with tile.TileContext(nc) as tc:
        with (
            tc.tile_pool(name="xp", bufs=XBUFS) as xp,
            tc.tile_pool(name="singles", bufs=1) as singles,
            tc.tile_pool(name="osb", bufs=4) as osb,
            tc.tile_pool(name="psum", bufs=1, space="PSUM") as psum,
        ):
            # ---------------- loads ----------------
            # hw (sync) ring order: wbf then xf0 (the first STT needs only
            # these), then the rest of the setup, then the f32 x stream.
            # sw (gpsimd) ring carries only the bf16 cast x stream.
            w_bc_f = singles.tile([P, E], F32)
            nc.sync.dma_start(out=w_bc_f, in_=wbf[:, :])

            # t = p*NCH + n; per-partition reads are NCH rows = 16 KiB
            xr = x[:, :, :].rearrange("b (p n) e -> b p n e", p=P, n=NCH)

            def load_x(b):
                if b % 2 == 0:
                    xt = xp.tile([P, NCH, E], F32R, tag="xf", name=f"xf{b}")
                    nc.sync.dma_start(out=xt, in_=xr[b].bitcast(F32R))
                else:
                    xt = xp.tile([P, NCH, E], BF16, tag="xb", name=f"xb{b}")
                    nc.gpsimd.dma_start(out=xt, in_=xr[b])
                return xt

            xts = [load_x(0), load_x(1)]

            w_bc = singles.tile([P, E], BF16)
            nc.sync.dma_start(out=w_bc, in_=wbb[:, :])
            bias_pt = singles.tile([P, NCH], F32)
            nc.sync.dma_start(out=bias_pt, in_=bias[:, :])
            cT = singles.tile([P, NE, BL], BF16)
            nc.sync.dma_start(out=cT, in_=cT_in[:, :, :])
            u_bf = singles.tile([P, NE, TL], BF16)
            nc.sync.dma_start(out=u_bf, in_=U_in[:, :, :])
            for b in range(2, min(2 * XBUFS, BL)):
                xts.append(load_x(b))

            ones_bf = singles.tile([P, 1], BF16)
            nc.vector.memset(ones_bf, 1.0)
            # f32r stationaries reject a free dim of 1; use two ones columns
            ones_fr = singles.tile([P, 2], F32R)
            nc.vector.memset(ones_fr.bitcast(F32), 1.0)

            # ---------------- ct = U.T @ cT (+bias-SHIFT) ----------------
            # One PSUM bank holds all NCH t-chunk accumulators as element-
            # disjoint regions; only the very first matmul uses start=True
            # (start clears the whole bank).
            ct_ps = psum.tile([P, NCH, BL], F32, tag="ctacc", bufs=1)
            for j in range(NE):
                for n in range(NCH):
                    nc.tensor.matmul(
                        ct_ps[:, n, :],
                        lhsT=u_bf[:, j, n::NCH],
                        rhs=cT[:, j, :],
                        start=(j == 0 and n == 0),
                        stop=(j == NE - 1 and n == NCH - 1),
                    )
            ct_all = singles.tile([P, NCH, BL], F32)
            for n in range(NCH):
                nc.scalar.activation(
                    out=ct_all[:, n, :],
                    in_=ct_ps[:, n, :],
                    func=mybir.ActivationFunctionType.Identity,
                    bias=bias_pt[:, n : n + 1],
                    scale=1.0,
                )

            # ---------------- main loop over batches ----------------
            sc_f = singles.tile([P, E], F32)  # DVE STT elementwise dump (f32)
            sc_b = singles.tile([P, E], BF16)  # and for bf16 batches
            et_big = singles.tile([P, BL, NCH], F32)
            # ev dtype matches the x parity (PE rejects mixed 32/16-bit)
            ev_f = singles.tile([P, BL // 2, NCH], F32R)
            ev_b = singles.tile([P, BL // 2, NCH], BF16)

            pending = []  # (b, ops) awaiting PSUM->SBUF copy
            pending_sb = []  # (b, out_sb) awaiting DRAM store

            def flush_copy():
                if pending:
                    pb, pops = pending.pop(0)
                    out_sb = osb.tile([1, 2, 512], F32, tag="osb")
                    nc.scalar.copy(out=out_sb, in_=pops)
                    pending_sb.append((pb, out_sb))

            def flush_store():
                if pending_sb:
                    pb, psb = pending_sb.pop(0)
                    nc.sync.dma_start(out=out[pb : pb + 1, :], in_=psb)

            half = BL * NCH // 2
            den_ps = psum.tile([2, BL * NCH], F32, tag="den", bufs=1)
            den_sb = singles.tile([1, BL * NCH], F32)

            for b in range(BL):
                if b == BL - 1:
                    # ev_f is complete once batch BL-2's exps are done; fold
                    # the even-batch denominator into the last batch's slot
                    # so the tail chain only holds the odd half
                    nc.tensor.matmul(
                        den_ps[:, 0:half],
                        lhsT=ones_fr,
                        rhs=ev_f[:, :, :],
                        start=True,
                        stop=False,
                    )
                    nc.scalar.copy(out=den_sb[:, 0:half], in_=den_ps[0:1, 0:half])
                xt = xts[b]
                is_f32 = b % 2 == 0
                sc = sc_f if is_f32 else sc_b
                ops = psum.tile([1, 2, 512], F32, tag="ops", bufs=2)
                for n in range(NCH):
                    nc.vector.scalar_tensor_tensor(
                        out=sc,
                        in0=xt[:, n, :].bitcast(F32) if is_f32 else xt[:, n, :],
                        scalar=0.0,
                        in1=w_bc_f if is_f32 else w_bc,
                        op0=mybir.AluOpType.add,
                        op1=mybir.AluOpType.mult,
                        accum_out=et_big[:, b, n : n + 1],
                    )
                    # ev = exp(et + ct + bias - SHIFT)
                    ev = (ev_f if is_f32 else ev_b)[:, b // 2, n : n + 1]
                    nc.scalar.activation(
                        out=ev,
                        in_=et_big[:, b, n : n + 1],
                        func=mybir.ActivationFunctionType.Exp,
                        bias=ct_all[:, n, b : b + 1],
                        scale=1.0,
                    )
                    for h in range(2):
                        nc.tensor.matmul(
                            ops[:, h, :],
                            lhsT=ev,
                            rhs=xt[:, n, h * 512 : (h + 1) * 512],
                            start=(n == 0),
                            stop=(n == NCH - 1),
                        )
                # tails of earlier batches (keeps the in-order ACT/Sync
                # queues free of head-of-line waits on this batch's work)
                flush_copy()
                flush_store()
                pending.append((b, ops))
                bn = b + 2 * XBUFS
                if bn < BL:
                    xts.append(load_x(bn))
            while pending or pending_sb:
                flush_copy()
                flush_store()

            # ---------------- denominator: odd half ----------------------
            # cols [0:64) = even-batch chunk sums (index (b//2)*NCH+n),
            # cols [64:128) = odd-batch chunk sums; host unscrambles
            nc.tensor.matmul(
                den_ps[0:1, half : BL * NCH],
                lhsT=ones_bf,
                rhs=ev_b[:, :, :],
                start=False,
                stop=True,
            )
            nc.scalar.copy(
                out=den_sb[:, half : BL * NCH], in_=den_ps[0:1, half : BL * NCH]
            )
            nc.sync.dma_start(out=den_out[:, :], in_=den_sb)

    nc.compile()
    return nc


def _get_exec():
    """Build the Bass program once and return (nc, in_names, out_names,
    zero_shapes, jitted _body). The multi-device shard_map path hangs through
    the axon tunnel, so we run 8 independent single-device executions
    instead (the kernel has no collectives)."""
    if "exec" in _CACHE:
        return _CACHE["exec"]

    import jax
    from concourse import bass2jax, mybir as _mybir

    bass2jax.install_neuronx_cc_hook()
    nc = build_bass()

    in_names, out_names, out_avals, zero_shapes = [], [], [], []
    for alloc in nc.m.functions[0].allocations:
        if not isinstance(alloc, _mybir.MemoryLocationSet):
            continue
        name = alloc.memorylocations[0].name
        if alloc.kind == "ExternalInput":
            in_names.append(name)
        elif alloc.kind == "ExternalOutput":
            out_names.append(name)
            shape = tuple(alloc.tensor_shape)
            dtype = _mybir.dt.np(alloc.dtype)
            out_avals.append(jax.core.ShapedArray(shape, dtype))
            zero_shapes.append((shape, dtype))
    n_params = len(in_names)
    all_names = in_names + out_names
    donate = tuple(range(n_params, n_params + len(out_names)))

    def _body(*args):
        outs = bass2jax._bass_exec_p.bind(
            *args,
            out_avals=tuple(out_avals),
            in_names=tuple(all_names),
            out_names=tuple(out_names),
            lowering_input_output_aliases=(),
            sim_require_finite=True,
            sim_require_nnan=True,
            nc=nc,
        )
        return tuple(outs)

    jitted = jax.jit(_body, donate_argnums=donate, keep_unused=True)
    _CACHE["exec"] = (nc, in_names, out_names, zero_shapes, jitted)
    return _CACHE["exec"]


def make_in_maps(x, c, W, b, U):
    """Per-core input dicts (full f32 inputs). Core k = ts*B_SHARD + bs.
    Small operands are pre-broadcast / pre-arranged / pre-cast on the host
    so the kernel does no on-chip setup work."""
    import ml_dtypes

    bf16 = ml_dtypes.bfloat16
    x = np.ascontiguousarray(x, dtype=np.float32)
    c = np.ascontiguousarray(c, dtype=np.float32)
    W = np.ascontiguousarray(W, dtype=np.float32)
    b = np.ascontiguousarray(b, dtype=np.float32)
    U = np.ascontiguousarray(U, dtype=np.float32)

    wbf = np.ascontiguousarray(np.broadcast_to(W[:, 0], (P, E)), dtype=np.float32)
    wbb = wbf.astype(bf16)
    maps = []
    for k in range(NCORES):
        ts, bs = divmod(k, B_SHARD)
        tsl = slice(ts * TL, (ts + 1) * TL)
        bsl = slice(bs * BL, (bs + 1) * BL)
        # bias[t] at [p, n] for t = p*NCH + n, with the exp shift folded in
        bias_arr = (b[tsl, 0] - SHIFT).reshape(P, NCH).astype(np.float32)
        # cT[e, b] = c[b, e] at [p, j, b] for e = p*NE + j
        ct_arr = np.ascontiguousarray(
            c[bsl].T.reshape(P, NE, BL), dtype=np.float32
        ).astype(bf16)
        # U at [p, j, t] for e = p*NE + j
        u_arr = np.ascontiguousarray(U[:, tsl].reshape(P, NE, TL)).astype(bf16)
        maps.append(
            {
                "x": np.ascontiguousarray(x[bsl, tsl, :]),
                "wbf": wbf,
                "wbb": wbb,
                "b": bias_arr,
                "ct": ct_arr,
                "u": u_arr,
            }
        )
    return maps


def combine(results):
    """Sum per-core partial outputs/denominators and normalize (f64)."""
    out = np.zeros((B, E), dtype=np.float64)
    den = np.zeros((B,), dtype=np.float64)
    half = BL * NCH // 2
    for k, res in enumerate(results):
        ts, bs = divmod(k, B_SHARD)
        bsl = slice(bs * BL, (bs + 1) * BL)
        out[bsl] += res["out"].astype(np.float64)
        raw = res["den"][0].astype(np.float64)
        dloc = np.zeros((BL,), dtype=np.float64)
        dloc[0::2] = raw[:half].reshape(BL // 2, NCH).sum(axis=1)
        dloc[1::2] = raw[half:].reshape(BL // 2, NCH).sum(axis=1)
        den[bsl] += dloc
    return (out / den[:, None]).astype(np.float32)


def kernel(x, c, W, b, U, trace=False, sequential=None):
    import jax

    nc, in_names, out_names, zero_shapes, jitted = _get_exec()

    if sequential is None:
        sequential = os.environ.get("BASS_KERNEL_SEQUENTIAL", "0") == "1"

    devices = jax.devices()[:NCORES]
    in_maps = make_in_maps(x, c, W, b, U)

    def _dispatch(k, dev):
        per_core = dict(in_maps[k])
        if nc.partition_id_tensor is not None:
            pid = nc.partition_id_tensor
            per_core[pid.name] = np.full(pid.shape, k, dtype=mybir.dt.np(pid.dtype))
        args = [
            jax.device_put(np.ascontiguousarray(per_core[n]), dev) for n in in_names
        ]
        args += [
            jax.device_put(np.zeros(shape, dtype), dev) for shape, dtype in zero_shapes
        ]
        return jitted(*args)

    parts = []
    if sequential:
        for k, dev in enumerate(devices):
            outs = _dispatch(k, dev)
            parts.append({name: np.asarray(outs[i]) for i, name in enumerate(out_names)})
    else:
        futures = [_dispatch(k, dev) for k, dev in enumerate(devices)]
        for outs in futures:
            parts.append({name: np.asarray(outs[i]) for i, name in enumerate(out_names)})
    return combine(parts)
